# revision 26
# baseline (speedup 1.0000x reference)
"""Trainium2 Bass kernel for nn_Attention (B=8, N=1024, C=768, H=12).

Strategy: pure data parallelism - one batch element per NeuronCore (8 cores,
zero collectives). Per core, a fused attention pipeline in bf16 on the
TensorEngine with f32 PSUM accumulation, built around PE tile co-streaming:

  - QK^T runs as row-tiled pairs (K=64 head halves on array rows 0:64 /
    64:128) which the PE streams concurrently -> 2 MMs per ~218ns slot.
  - PV, the qkv/out projections, the Z (softmax denominator) reduction and
    the bias add all run column-tiled (M=64 halves of the output partition
    range), which also co-streams. Full-array (128x128) matmuls are avoided
    entirely so the engine only ever alternates between the 64x128 row mode
    (QK) and the 128x64 col mode (everything else).
  - Z is computed with an ones[128,64] stationary -> the result arrives
    already broadcast across 64 partitions; reciprocal on DVE then yields a
    broadcast 1/Z tile, so normalization fuses into the PSUM->SBUF
    evacuation of the PV accumulator (single tensor_tensor multiply).
  - softmax without max-subtraction (scores provably small here).
  - bias is folded in as a K=128 matmul against an e0 selector matrix.

Layout notes: all matmuls contract over the partition dim; "T" suffixes mean
channel-on-partition layouts so no on-device transposes are ever needed.
"""

import numpy as np
import ml_dtypes

N = 1024  # tokens
C = 768  # channels
H = 12  # heads
D = 64  # head dim
NPAIR = 6  # head pairs (2 heads per 128-partition chunk)
CCH = 6  # C // 128 chunks
KC = 8  # key chunks of 128
TT = 8  # token tiles of 128
QH = 2  # query halves of 512
QW = 512

_CACHE = {}


def _build():
    import concourse.bacc as bacc
    import concourse.tile as tile
    import concourse.mybir as mybir

    dt = mybir.dt
    Alu = mybir.AluOpType
    Act = mybir.ActivationFunctionType

    nc = bacc.Bacc("TRN2", target_bir_lowering=False, debug=False, num_devices=8)

    xT_e = nc.declare_dram_parameter("xT", [C, N], dt.bfloat16, isOutput=False)
    wqT_e = nc.declare_dram_parameter("wqT", [C, C], dt.bfloat16, isOutput=False)
    wkT_e = nc.declare_dram_parameter("wkT", [C, C], dt.bfloat16, isOutput=False)
    wvT_e = nc.declare_dram_parameter("wvT", [C, C], dt.bfloat16, isOutput=False)
    wpT_e = nc.declare_dram_parameter("wpT", [C, C], dt.bfloat16, isOutput=False)
    ones_e = nc.declare_dram_parameter("ones", [128, 128], dt.bfloat16, isOutput=False)
    e0_e = nc.declare_dram_parameter("e0", [128, 128], dt.bfloat16, isOutput=False)
    bias2_e = nc.declare_dram_parameter("bias2", [128, C], dt.bfloat16, isOutput=False)
    y_e = nc.declare_dram_parameter("y", [N, C], dt.float32, isOutput=True)

    with tile.TileContext(nc) as tc:
        with (
            tc.tile_pool(name="sbw", bufs=1) as sbw,
            tc.tile_pool(name="sbqk", bufs=1) as sbqk,
            tc.tile_pool(name="sbp", bufs=2) as sbp,
            tc.tile_pool(name="sbo", bufs=4) as sbo,
            tc.tile_pool(name="ps_s", bufs=2, space="PSUM") as ps_s,
            tc.tile_pool(name="ps_pv", bufs=1, space="PSUM") as ps_pv,
            tc.tile_pool(name="ps_x", bufs=2, space="PSUM") as ps_x,
        ):
            # ---------------- persistent SBUF tensors + input DMAs ----------
            xT = sbw.tile([128, CCH, N], dt.bfloat16, tag="xT")
            wq = sbw.tile([128, CCH, C], dt.bfloat16, tag="wq")
            wk = sbw.tile([128, CCH, C], dt.bfloat16, tag="wk")
            wv = sbw.tile([128, CCH, C], dt.bfloat16, tag="wv")
            wp = sbw.tile([128, CCH, C], dt.bfloat16, tag="wp")
            ones = sbw.tile([128, 128], dt.bfloat16, tag="ones")
            e0 = sbw.tile([128, 128], dt.bfloat16, tag="e0")
            bias2 = sbw.tile([128, C], dt.bfloat16, tag="bias2")
            # priority order: small tensors first (warm-keeper matmuls need
            # `ones` asap), then xT+wq (gate the first dose), wk, wv, wp
            # last (only needed by the output proj)
            # descriptor issue is ~650ns per dma_start per queue; spread the
            # input loads across four otherwise-idle engine queues so the
            # critical tensors (xT, wq, wk) issue in parallel
            nc.sync.dma_start(ones[:], ones_e[:])
            for c in range(CCH):
                sl = slice(c * 128, (c + 1) * 128)
                nc.sync.dma_start(xT[:, c, :], xT_e[sl, :])
                nc.scalar.dma_start(wq[:, c, :], wqT_e[sl, :])
                nc.gpsimd.dma_start(wk[:, c, :], wkT_e[sl, :])
            for c in range(CCH):
                sl = slice(c * 128, (c + 1) * 128)
                nc.gpsimd.dma_start(wv[:, c, :], wvT_e[sl, :])
            nc.gpsimd.dma_start(e0[:], e0_e[:])
            nc.gpsimd.dma_start(bias2[:], bias2_e[:])
            for c in range(CCH):
                sl = slice(c * 128, (c + 1) * 128)
                nc.sync.dma_start(wp[:, c, :], wpT_e[sl, :])

            qT = sbqk.tile([128, NPAIR, N], dt.bfloat16, tag="qT")
            kT = sbqk.tile([128, NPAIR, N], dt.bfloat16, tag="kT")
            v = sbqk.tile([128, TT, C], dt.bfloat16, tag="v")
            outNT = sbqk.tile([128, NPAIR, N], dt.bfloat16, tag="outNT")

            ST = {}  # per-pair live state

            # ---------------- helpers ---------------------------------------
            # keep-alive loads: fill DMA-paced idle stretches so the PE_HAM
            # clock gate stays open. LDWEIGHTS-only: counts as PE activity,
            # needs no PSUM, and the next real matmul reloads its own weights
            # anyway (col-mode tile so no tiling-mode switch).
            def warm(n):
                for _ in range(n):
                    nc.tensor.ldweights(ones[:, 0:64], tile_position=(0, 0))
            def dose(j, which, qh, warm_each=0):
                """col-split projection of q or k for pair j, query half qh"""
                w_sb, dst = (wq, qT) if which == "q" else (wk, kT)
                qs = slice(qh * QW, (qh + 1) * QW)
                ps = ps_x.tile([128, QW], dt.float32, tag="x", name="dose")
                lo = slice(j * 128, j * 128 + 64)
                hi = slice(j * 128 + 64, (j + 1) * 128)
                for cc in range(CCH):
                    nc.tensor.matmul(
                        ps[0:64, :], w_sb[:, cc, lo], xT[:, cc, qs],
                        start=(cc == 0), stop=(cc == CCH - 1),
                        skip_group_check=True,
                    )
                    nc.tensor.matmul(
                        ps[64:128, :], w_sb[:, cc, hi], xT[:, cc, qs],
                        start=(cc == 0), stop=(cc == CCH - 1),
                        skip_group_check=True,
                    )
                    if warm_each:
                        warm(warm_each)
                nc.vector.tensor_copy(dst[:, j, qs], ps[:])

            def v_half(t, hs_i):
                """col-split v projection for token tile t, channel half hs_i"""
                hs = slice(0, 512) if hs_i == 0 else slice(512, C)
                w_ = 512 if hs_i == 0 else C - 512
                ps = ps_x.tile([128, w_], dt.float32, tag="x", name="vp")
                lo = slice(t * 128, t * 128 + 64)
                hi = slice(t * 128 + 64, (t + 1) * 128)
                for cc in range(CCH):
                    nc.tensor.matmul(
                        ps[0:64, :], xT[:, cc, lo], wv[:, cc, hs],
                        start=(cc == 0), stop=(cc == CCH - 1),
                        skip_group_check=True,
                    )
                    nc.tensor.matmul(
                        ps[64:128, :], xT[:, cc, hi], wv[:, cc, hs],
                        start=(cc == 0), stop=(cc == CCH - 1),
                        skip_group_check=True,
                    )
                nc.vector.tensor_copy(v[:, t, hs], ps[:])

            def qk_kc(j, kc, defer_z=False):
                """row-tiled QK + exp + running-z for (pair j, key chunk kc).
                With defer_z the DVE z-accumulate is returned as a thunk so
                the previous pair's norm can be queued on DVE ahead of it."""
                if kc == 0:
                    ST[j] = dict(
                        P=sbp.tile([128, KC, 2 * N], dt.bfloat16, tag="P",
                                   name="P"),
                        zab=sbp.tile([128, 2 * N], dt.bfloat16, tag="zab",
                                     name="zab"),
                    )
                st = ST[j]
                ks = slice(kc * 128, (kc + 1) * 128)
                s_a = ps_s.tile([128, N], dt.float32, tag="s", name="sa")
                s_b = ps_s.tile([128, N], dt.float32, tag="s", name="sb")
                for qh in range(QH):
                    qs = slice(qh * QW, (qh + 1) * QW)
                    nc.tensor.matmul(s_a[:, qs], kT[0:64, j, ks],
                                     qT[0:64, j, qs], skip_group_check=True)
                    nc.tensor.matmul(s_b[:, qs], kT[64:128, j, ks],
                                     qT[64:128, j, qs], skip_group_check=True)
                nc.scalar.activation(st["P"][:, kc, 0:N], s_a[:], Act.Exp)
                nc.scalar.activation(st["P"][:, kc, N:2 * N], s_b[:], Act.Exp)

                def zacc():
                    if kc == 0:
                        nc.vector.tensor_copy(st["zab"][:], st["P"][:, 0, :])
                    else:
                        nc.vector.tensor_tensor(
                            st["zab"][:], st["zab"][:], st["P"][:, kc, :],
                            Alu.add
                        )

                if defer_z:
                    return zacc
                zacc()
                return None

            def pv_kc(j, kc):
                """col-tiled PV accumulation for (pair j, key chunk kc)"""
                st = ST[j]
                if kc == 0:
                    st["outT"] = ps_pv.tile([128, N], dt.float32, tag="pv",
                                            name="outT")
                outT = st["outT"]
                cA = slice(j * 128, j * 128 + 64)
                cB = slice(j * 128 + 64, (j + 1) * 128)
                for qh in range(QH):
                    qs = slice(qh * QW, (qh + 1) * QW)
                    nc.tensor.matmul(
                        outT[0:64, qs], v[:, kc, cA], st["P"][:, kc, qs],
                        start=(kc == 0), stop=(kc == KC - 1),
                        skip_group_check=True,
                    )
                    nc.tensor.matmul(
                        outT[64:128, qs], v[:, kc, cB],
                        st["P"][:, kc, N:N + QW] if qh == 0
                        else st["P"][:, kc, N + QW:2 * N],
                        start=(kc == 0), stop=(kc == KC - 1),
                        skip_group_check=True,
                    )

            def z_half(j, qh):
                """Z reduction for pair j, query half qh: broadcast col pair"""
                st = ST[j]
                if qh == 0:
                    st["Rbc"] = sbo.tile([128, N], dt.float32, tag="Rbc",
                                         name="Rbc")
                qs = slice(qh * QW, (qh + 1) * QW)
                zps = ps_x.tile([128, QW], dt.float32, tag="x", name="zps")
                nc.tensor.matmul(zps[0:64, :], ones[:, 0:64],
                                 st["zab"][:, qs], skip_group_check=True)
                nc.tensor.matmul(zps[64:128, :], ones[:, 64:128],
                                 st["zab"][:, N + qh * QW:N + (qh + 1) * QW],
                                 skip_group_check=True)
                nc.vector.reciprocal_approx_fast(st["Rbc"][:, qs], zps[:])

            def norm(j):
                """outNT[:, j, :] = outT * (1/Z): fused evac + normalize"""
                st = ST.pop(j)
                nc.vector.tensor_tensor(
                    outNT[:, j, :], st["outT"][:], st["Rbc"][:], Alu.mult
                )

            def proj_part(t, hs_i, jmax, pool=None, tag="s"):
                """col-split output projection for token tile t: bias + pairs
                0..jmax accumulating into a persistent psum tile"""
                ps = (pool or ps_s).tile([128, C], dt.float32, tag=tag,
                                         name="proj")
                lo = slice(t * 128, t * 128 + 64)
                hi = slice(t * 128 + 64, (t + 1) * 128)
                for hs_ii in range(2):
                    hs = slice(0, 512) if hs_ii == 0 else slice(512, C)
                    nc.tensor.matmul(ps[0:64, hs], e0[:, 0:64], bias2[:, hs],
                                     start=True, stop=False,
                                     skip_group_check=True)
                    nc.tensor.matmul(ps[64:128, hs], e0[:, 64:128],
                                     bias2[:, hs], start=True, stop=False,
                                     skip_group_check=True)
                    for j in range(jmax + 1):
                        nc.tensor.matmul(
                            ps[0:64, hs], outNT[:, j, lo], wp[:, j, hs],
                            start=False, stop=False, skip_group_check=True,
                        )
                        nc.tensor.matmul(
                            ps[64:128, hs], outNT[:, j, hi], wp[:, j, hs],
                            start=False, stop=False, skip_group_check=True,
                        )
                return ps

            def proj_fin(t, ps, jmin):
                """finish proj tile t: pairs jmin..5, evacuate + DMA out"""
                lo = slice(t * 128, t * 128 + 64)
                hi = slice(t * 128 + 64, (t + 1) * 128)
                for hs_ii in range(2):
                    hs = slice(0, 512) if hs_ii == 0 else slice(512, C)
                    for j in range(jmin, NPAIR):
                        nc.tensor.matmul(
                            ps[0:64, hs], outNT[:, j, lo], wp[:, j, hs],
                            start=False, stop=(j == NPAIR - 1),
                            skip_group_check=True,
                        )
                        nc.tensor.matmul(
                            ps[64:128, hs], outNT[:, j, hi], wp[:, j, hs],
                            start=False, stop=(j == NPAIR - 1),
                            skip_group_check=True,
                        )
                y_sb = sbo.tile([128, C], dt.float32, tag="y")
                nc.vector.tensor_copy(y_sb[:, 0:512], ps[:, 0:512])
                nc.sync.dma_start(y_e[t * 128:(t + 1) * 128, 0:512],
                                  y_sb[:, 0:512])
                nc.vector.tensor_copy(y_sb[:, 512:C], ps[:, 512:C])
                nc.sync.dma_start(y_e[t * 128:(t + 1) * 128, 512:C],
                                  y_sb[:, 512:C])

            # ---------------- emission: software-pipelined schedule ---------
            # warmup: doses for pairs 0 and 1 (DMA-paced); keep-alive MMs
            # between sub-groups so the HAM clock gate opens early
            warm(8)
            for j in (0, 1):
                for which in ("q", "k"):
                    for qh in range(QH):
                        dose(j, which, qh, warm_each=1)
                        warm(6)

            # step 0: QK(0) + v tiles + q-doses for pair 2. Even kc emit the
            # row-mode QK first, odd kc last, so col-mode runs merge across
            # the slot boundary (1 mode switch per slot instead of 2).
            for kc in range(KC):
                if kc % 2 == 0:
                    qk_kc(0, kc)
                if kc < 6:
                    v_half(kc, 0)
                    warm(2)  # covers v evac before the next x-buffer user
                    v_half(kc, 1)
                if kc in (6, 7):
                    dose(2, "q", kc - 6)
                    warm(2)
                if kc % 2 == 1:
                    qk_kc(0, kc)

            # steps 1..5; pv is emitted after the slot's other col-mode work
            # so the tensor queue has independent work while pv waits on the
            # previous pair's norm to release the accumulator. K-doses of
            # pair s+1 at kc 0/2, q-doses of pair s+2 at kc 4/6.
            for s in range(1, 6):
                jq = s  # pair for QK
                jp = s - 1  # pair for PV
                zlast = None
                for kc in range(KC):
                    if kc % 2 == 0:
                        qk_kc(jq, kc)
                    if s == 1 and kc in (0, 1):
                        v_half(6 + kc, 0)
                        v_half(6 + kc, 1)
                    if s <= 4 and kc in (0, 2):
                        dose(s + 1, "k", kc // 2)
                    if s <= 3 and kc in (4, 6):
                        dose(s + 2, "q", (kc - 4) // 2)
                    if kc == 3:
                        z_half(jp, 0)
                    elif kc == 5:
                        z_half(jp, 1)
                    pv_kc(jp, kc)
                    if s >= 4:
                        warm(2)
                    if kc % 2 == 1:
                        zlast = qk_kc(jq, kc, defer_z=(kc == KC - 1))
                # norm goes on the DVE queue ahead of the last z-accumulate
                # so the pv accumulator frees before the step boundary
                norm(jp)
                zlast()

            # step 6: PV(5) + Z(5) + proj partials for tiles 0 and 1
            proj_ps = {}
            for kc in range(KC):
                if kc == 3:
                    z_half(5, 0)
                elif kc == 5:
                    z_half(5, 1)
                pv_kc(5, kc)
                if kc == 2:
                    proj_ps[0] = proj_part(0, 0, 4)
                warm(2)
            norm(5)

            # tail: part(1) covers the norm(5) latency (its s-buffer is
            # already free); then 3-buffer rotation (s, s, pv) so a partial
            # never waits on the previous finisher's evacuation
            proj_ps[1] = proj_part(1, 0, 4)
            proj_fin(0, proj_ps[0], 5)
            for t in range(2, TT):
                if t in (2, 5, 7):
                    proj_ps[t] = proj_part(t, 0, 4, pool=ps_pv, tag="pv")
                else:
                    proj_ps[t] = proj_part(t, 0, 4)
                proj_fin(t - 1, proj_ps[t - 1], 5)
            proj_fin(7, proj_ps[7], 5)

    nc.compile()
    return nc


def _built():
    if "nc" not in _CACHE:
        _CACHE["nc"] = _build()
    return _CACHE["nc"]


def kernel(x, w_qkv, w_proj, b_proj):
    from concourse.bass_utils import run_bass_kernel_spmd

    nc = _built()
    bf16 = ml_dtypes.bfloat16
    scale = np.float32(D**-0.5)

    wqT = np.ascontiguousarray((w_qkv[0:C].astype(np.float32) * scale).T).astype(bf16)
    wkT = np.ascontiguousarray(w_qkv[C:2 * C].astype(np.float32).T).astype(bf16)
    wvT = np.ascontiguousarray(w_qkv[2 * C:3 * C].astype(np.float32).T).astype(bf16)
    wpT = np.ascontiguousarray(w_proj.astype(np.float32).T).astype(bf16)
    ones = np.ones((128, 128), dtype=bf16)
    e0 = np.zeros((128, 128), dtype=bf16)
    e0[0, :] = 1
    bias2 = np.zeros((128, C), dtype=np.float32)
    bias2[0, :] = np.asarray(b_proj, dtype=np.float32)
    bias2 = bias2.astype(bf16)

    x = np.asarray(x, dtype=np.float32)
    in_maps = []
    for b in range(8):
        xTb = np.ascontiguousarray(x[b].T).astype(bf16)
        in_maps.append(
            dict(
                xT=xTb,
                wqT=wqT,
                wkT=wkT,
                wvT=wvT,
                wpT=wpT,
                ones=ones,
                e0=e0,
                bias2=bias2,
            )
        )

    res = run_bass_kernel_spmd(nc, in_maps, list(range(8)))
    out = np.stack([res.results[b]["y"] for b in range(8)], axis=0)
    return out.astype(np.float32)
